# revision 70
# baseline (speedup 1.0000x reference)
"""MoE gate (softmax + top-8 + renormalize) Trainium2 Bass kernel.

Problem: hidden_states [4, 4096, 2048] f32, weight [64, 2048] f32.
  logits = x @ W.T            [16384, 64]
  scores = softmax(logits)
  topk_w, topk_idx = top_k(scores, 8);  topk_w /= topk_w.sum(-1)

Key identities used:
  - top-8 indices of softmax(logits) == top-8 indices of logits
  - renormalized top-8 softmax probs == softmax over just the top-8 logits
    (the global softmax denominator cancels), so the full [T,64] softmax is
    never materialized.

Sharding: tokens split 2048-per-core across 8 NeuronCores; weight replicated.
The token shard of x is transposed on the HOST (numpy) so the device reads
x^T with H on partitions — the layout the PE's contraction needs — at full
contiguous DMA bandwidth. No on-device transposes of the big tensor.

Per core device program:
  - load W^T [2048, 64] once (512 KB)
  - preload the whole x^T shard (16 MB) into SBUF with a few large
    contiguous DMAs (128 KB/partition out of 192)
  - two half-passes over tokens (8 PSUM banks each): per half,
    16 H-tile matmuls per token-tile accumulate logits [128t, 64e] in PSUM
    (lhsT = x^T block [128h, 128t] stationary, rhs = W^T tile [128h, 64e])
  - epilogue per 128-token tile: copy PSUM->SBUF, hardware top-8
    (InstMax + InstMaxIndex), exp (ACT, with per-partition -max bias and
    fused sum), reciprocal, scale -> weights; stage results
  - two output DMAs: weights [2048, 8] f32, indices [2048, 8] u32

Toolchain constraint baked into the structure: this walrus build allows at
most ONE sync-wait command per instruction, so the program is arranged so
no instruction ever needs two (single monotonic HWDGE sem lane, no SBUF
slot reuse, per-engine SP catch-up nops before the kernel-tail drain).
"""

import sys

if "/opt/trn_rl_repo" not in sys.path:
    sys.path.insert(0, "/opt/trn_rl_repo")

import numpy as np

N_CORES = 8
T_TOTAL = 16384
T_CORE = T_TOTAL // N_CORES   # 2048 tokens per core
H = 2048
E = 64
TOP_K = 8

HT = H // 128                 # 16 contraction tiles
NT = T_CORE // 128            # 16 token-tiles of 128
# Activation-load plan: (ring, start_h, n_h_tiles) in h order (the PE
# consumes h in order). Chunks alternate between the SP HWDGE ring and
# the gpsimd SWDGE ring so the two descriptor-generation paths overlap
# and transfers interleave at packet granularity; small first chunks let
# the PE start early.
LOAD_PLAN = (
    ("gpsimd", 0, 1), ("gpsimd", 1, 1), ("gpsimd", 2, 2), ("gpsimd", 4, 2),
    ("sync", 6, 4), ("sync", 10, 4), ("sync", 14, 2),
)

_cached = {}


def _build_program(n_halves=2, timing=False):
    import concourse.bass as bass
    import concourse.tile as tile
    import concourse.tile_sem_assignment as tsa
    from concourse import mybir

    # Tile round-robins DMA completions across several sem lanes, which can
    # leave one instruction waiting on two lanes. All our DMAs issue from
    # a single FIFO ring per engine, so collapsing each ring to one lane is
    # lossless and every wait becomes a single monotonic sem-ge condition.
    # All HWDGE loads share one monotonic sem lane (they issue from the
    # single SP FIFO ring, so one lane is lossless and every consumer wait
    # is a single sem-ge condition). SWDGE keeps its default lane count:
    # with our 6 SWDGE DMAs each landing on its own lane, the output
    # stores see pristine lanes and need no lane catch-up wait.
    tsa.NUM_HWDGE_SEMS = 1

    f32 = mybir.dt.float32
    u32 = mybir.dt.uint32

    nc = bass.Bass()
    # Timing variants use Internal DRAM for the big inputs so the axon
    # runner ships no activation data per call; kernel timing is
    # data-independent.
    in_kind = "Internal" if timing else "ExternalInput"
    xt = nc.dram_tensor("xt", [H, T_CORE], f32, kind=in_kind)
    # wt arrives host-prearranged in p-major [128, HT, E] layout so the
    # load is one fully-contiguous 4KB-per-partition DMA (128 descriptors)
    # on the Pool ring, ahead of the x chunks.
    wt = nc.dram_tensor("wt", [128, HT, E], f32, kind=in_kind)
    out_w = nc.dram_tensor("out_w", [T_CORE, TOP_K], f32, kind="ExternalOutput")
    out_i = nc.dram_tensor("out_i", [T_CORE, TOP_K], u32, kind="ExternalOutput")

    with tile.TileContext(nc) as tc:
        with (
            tc.tile_pool(name="wpool", bufs=1) as wpool,
            tc.tile_pool(name="xpool", bufs=1) as xpool,
            tc.tile_pool(name="psum", bufs=8, space="PSUM") as psum,
            # One buffer per token-tile: epilogue tiles are tiny and slot
            # reuse would add second sync-waits.
            tc.tile_pool(name="epi", bufs=NT) as epi,
            tc.tile_pool(name="stage", bufs=1) as stage,
        ):
            wt_sb = wpool.tile([128, HT, E], f32)
            nc.gpsimd.dma_start(wt_sb[:], wt[:])

            last_per_engine = {}
            if n_halves > 0:
                stage_w = stage.tile([128, NT, TOP_K], f32)
                stage_i = stage.tile([128, NT, TOP_K], u32)

                # Preload the full x^T shard into one big SBUF tile
                # (subtile deps let each matmul wait only on the DMA that
                # wrote its H-tiles). DMAs alternate between the SP HWDGE
                # ring and the gpsimd SWDGE ring: each ring's completions
                # land on its own (FIFO-ordered) sem lane, and the two
                # rings' fixed costs overlap.
                xp = xpool.tile([128, HT, T_CORE], f32)
                for di, (ring, h0, hpd) in enumerate(LOAD_PLAN):
                    eng = nc.sync if ring == "sync" else nc.gpsimd
                    # gpsimd loads each land on their own SWDGE sem lane;
                    # track every one so an SP catch-up nop can observe
                    # each lane before the tail drain.
                    key = "dma_in" if ring == "sync" else f"dma_in_sw{di}"
                    last_per_engine[key] = eng.dma_start(
                        xp[:, h0 : h0 + hpd, :],
                        xt[128 * h0 : 128 * (h0 + hpd), :].rearrange(
                            "(a p) t -> p a t", p=128
                        ),
                    )

                # All 16 logits accumulators [128, 64] live in 2 PSUM
                # banks: one accumulation group per bank (start clears the
                # bank; first write to each region lands via has_written).
                # 8 banks x 2 token-tiles: the DVE epilogue for a bank can
                # only start once the PE stops writing that bank (bank-
                # overlap serialization), so finer bank granularity lets
                # epilogue chains overlap the last matmul round.
                TPB = NT // 8  # token-tiles per bank
                ps_banks = [
                    psum.tile([128, TPB, E], f32, tag="ps", name=f"ps_{b}")
                    for b in range(8)
                ]
                # wt and h0 arrive on different SWDGE lanes; a throwaway
                # 1x1 matmul absorbs the h0-lane wait so the first real
                # matmul only waits on the wt lane (one-wait limit). Its
                # garbage write is overwritten by the real start=True
                # matmul.
                dmy = nc.tensor.matmul(
                    ps_banks[0][0:1, 0, 0:1],
                    xp[0:1, 0, 0:1],
                    xp[0:1, 0, 0:1],
                    start=True,
                    stop=True,
                )
                first_mm = None
                for h in range(HT):
                    for tt in range(NT):
                        last_per_engine["pe"] = nc.tensor.matmul(
                            ps_banks[tt // TPB][:, tt % TPB, :],
                            xp[:, h, 128 * tt : 128 * (tt + 1)],
                            wt_sb[:, h, :],
                            start=(h == 0 and tt % TPB == 0),
                            stop=(h == HT - 1 and tt % TPB == TPB - 1),
                        )
                        if first_mm is None:
                            first_mm = last_per_engine["pe"]
                            tile.add_dep_helper(
                                first_mm.ins, dmy.ins, sync=False,
                                reason="order real MMs after wait-collector",
                            )
                for tt in range(NT):
                    s = ps_banks[tt // TPB][:, tt % TPB, :]
                    vals = epi.tile([128, TOP_K], f32)
                    nc.vector.max(vals[:], s[:])
                    nc.vector.max_index(stage_i[:, tt, :], vals[:], s[:])
                    negm = epi.tile([128, 1], f32)
                    nc.vector.tensor_scalar_mul(negm[:], vals[:, 0:1], -1.0)
                    ex = epi.tile([128, TOP_K], f32)
                    ssum = epi.tile([128, 1], f32)
                    last_per_engine["act"] = nc.scalar.activation(
                        ex[:],
                        vals[:],
                        mybir.ActivationFunctionType.Exp,
                        bias=negm[:],
                        scale=1.0,
                        accum_out=ssum[:],
                    )
                    rcp = epi.tile([128, 1], f32)
                    nc.vector.reciprocal(rcp[:], ssum[:])
                    last_per_engine["dve"] = nc.vector.tensor_scalar_mul(
                        stage_w[:, tt, :], ex[:], rcp[:]
                    )

                # Output stores go out on SWDGE lanes, so each carries its
                # DVE data dep as the sole wait (their lanes' prior traffic
                # is already sem-ordered ahead of them).
                last_per_engine["dma_w"] = nc.gpsimd.dma_start(
                    out_w.rearrange("(a p) k -> p a k", p=128), stage_w[:]
                )
                last_per_engine["dma_i"] = nc.gpsimd.dma_start(
                    out_i.rearrange("(a p) k -> p a k", p=128), stage_i[:]
                )

            # The kernel-tail drain on SP must catch its clock up to every
            # other proc; walrus only allows one sync-wait per instruction,
            # so stage the catch-up through single-dep SP nops first.
            for key, target in last_per_engine.items():
                nop = nc.sync.nop(hint=f"sp_catchup_{key}", nofuse=True)
                tile.add_dep_helper(
                    nop.ins, target.ins, sync=True,
                    reason=f"SP clock catch-up on {key}",
                )

    for f in nc.m.functions:
        for b in f.blocks:
            for inst in b.instructions:
                if inst.sync_info and len(inst.sync_info.on_wait) > 1:
                    if type(inst).__name__ != "InstDrain":
                        raise AssertionError(
                            f"{inst.name} ({type(inst).__name__}) has "
                            f"{len(inst.sync_info.on_wait)} waits"
                        )
    return nc


def _get_program(n_halves=2, timing=False):
    key = ("nc", n_halves, timing)
    if key not in _cached:
        _cached[key] = _build_program(n_halves, timing)
    return _cached[key]


def _make_in_maps(hidden_states, weight):
    x = np.asarray(hidden_states, dtype=np.float32).reshape(T_TOTAL, H)
    w = np.asarray(weight, dtype=np.float32)
    # p-major [128, HT, E]: wt[p, a, e] = weight[e, 128*a + p]
    wt = np.ascontiguousarray(
        w.T.reshape(H // 128, 128, E).transpose(1, 0, 2)
    )
    in_maps = []
    for i in range(N_CORES):
        xs = x[i * T_CORE : (i + 1) * T_CORE]
        in_maps.append({"xt": np.ascontiguousarray(xs.T), "wt": wt})
    return in_maps


def _gather(results):
    topk_w = np.concatenate([results[i]["out_w"] for i in range(N_CORES)], axis=0)
    topk_i = np.concatenate([results[i]["out_i"] for i in range(N_CORES)], axis=0)
    return topk_w.astype(np.float32), topk_i.astype(np.int32)


def kernel(hidden_states, weight):
    from concourse.bass_utils import run_bass_kernel_spmd

    nc = _get_program()
    in_maps = _make_in_maps(hidden_states, weight)
    res = run_bass_kernel_spmd(nc, in_maps, list(range(N_CORES)))
    return _gather(res.results)
